# revision 25
# baseline (speedup 1.0000x reference)
"""Trainium2 Bass kernel for nn_CudaFastWeightSumPerformerLayer.

Performer FAVOR+ fast-weight (causal linear attention) layer.
Sharding: 8 cores = 4 batches x 2 head-groups (4 heads each). Each core
computes qkv projection, prime features, and the chunked causal
linear-attention scan in bf16, emitting the normalized per-head attention
output (B,heads,L,dh) as one bf16 tensor. Host applies the (small) w_o
projection, residual, and LayerNorm in f32.

Math restructure (validated vs reference):
  - The FAVOR+ diag term exp(-0.5|x|^2) cancels in the normalized output,
    so features are just [exp(d), exp(-d)], d = (x * dh^-0.25) @ proj.
  - kp normalization (1/sum) is folded into V' columns; the attention
    denominator and the q-feature sum arrive as extra output columns via
    ones-columns in V' and the scan state.
  - ksum (per-key feature sum) is obtained for free as an extra column of
    the B matrix by appending a ones column to the qp chunk (129-col rhs).
  - out_final = out_raw / (denom_raw + eps * qsum).
Chunked scan (chunk=128): B[j,t] = kp_j . qp_t (masked j<=t),
  out_c = B^T @ V' + qp_c @ S;  S += kp_c^T @ V'.
All matmuls bf16 with f32 PSUM accumulation. I/O is bf16 and packed into
two input DMAs and one output DMA, each contiguous per partition.
"""

import numpy as np

L, DM, DH, M = 2048, 512, 64, 256
F = 2 * M          # 512 feature dim
NH = 8             # total heads
HPC = 4            # heads per core
B = 4
CH = 128           # scan chunk
NCH = L // CH      # 16
SCALE = DH ** -0.5
EPS_ATTN = 1e-5
EPS_LN = 1e-5
N_CORES = 8

# packed weight blob column offsets (bf16, per partition)
BLOB_WQK = 0            # [p, kc, 512]   kc in 0..3   (2048 cols)
BLOB_WV = 2048          # [p, kc, 256]                (1024 cols)
BLOB_PROJ = 3072        # [p, 256] proj duplicated on both 64-halves
BLOB_MASK = 3328        # [p, 128] upper-triangular mask (j<=t)
BLOB_COLS = 3456

_CACHE = {}


def _build_nc():
    import concourse.bacc as bacc
    import concourse.tile as tile
    from concourse import mybir

    f32 = mybir.dt.float32
    bf16 = mybir.dt.bfloat16
    AF = mybir.ActivationFunctionType
    ALU = mybir.AluOpType

    nc = bacc.Bacc("TRN2", target_bir_lowering=False, debug=False,
                   num_devices=N_CORES)

    hT_d = nc.dram_tensor("hT", [128, 4 * L], bf16, kind="ExternalInput")
    blob_d = nc.dram_tensor("blob", [128, BLOB_COLS], bf16,
                            kind="ExternalInput")
    part_d = nc.dram_tensor("part", [128, NCH * 4 * DH], bf16,
                            kind="ExternalOutput")

    with tile.TileContext(nc) as tc:
        from contextlib import ExitStack
        with ExitStack() as ctx:
            consts = ctx.enter_context(tc.tile_pool(name="consts", bufs=1))
            qkpool = ctx.enter_context(tc.tile_pool(name="qkpool", bufs=1))
            vpool = ctx.enter_context(tc.tile_pool(name="vpool", bufs=1))
            onorm = ctx.enter_context(tc.tile_pool(name="onorm", bufs=1))

            blob = consts.tile([128, BLOB_COLS], bf16, tag="blob",
                               name="blob")
            nc.sync.dma_start(out=blob, in_=blob_d[:, :])
            wqk_sb = [blob[:, BLOB_WQK + 512 * kc:BLOB_WQK + 512 * (kc + 1)]
                      for kc in range(4)]
            wv_sb = [blob[:, BLOB_WV + 256 * kc:BLOB_WV + 256 * (kc + 1)]
                     for kc in range(4)]
            proj_sb = blob[:, BLOB_PROJ:BLOB_PROJ + 256]
            masku_sb = blob[:, BLOB_MASK:BLOB_MASK + 128]

            # scan output, token-major: [p, chunk, 4 heads x 64] bf16
            on_all = onorm.tile([128, NCH * 4 * DH], bf16, tag="on",
                                name="on")

            # ---- phase 1: qkv projection (bf16) ----
            qk_sb = [qkpool.tile([128, L], bf16, tag=f"qk{m}", name=f"qk{m}")
                     for m in range(HPC)]
            # v, token-major: [p, chunk, 4 heads x 65] bf16; col 64 of each
            # head block is preset to 1.0 so V' = v_block * rk in one op
            v_all = vpool.tile([128, NCH * 4 * (DH + 1)], bf16, tag="v",
                               name="v")
            v_view = v_all[:].rearrange("p (c h f) -> p c h f", h=4, f=DH + 1)
            for mm in range(4):
                nc.vector.memset(v_view[:, :, mm, 64:65], 1.0)
            with tc.tile_pool(name="hTp", bufs=1) as hTp, \
                 tc.tile_pool(name="p1ps", bufs=2, space="PSUM") as p1ps, \
                 tc.tile_pool(name="p1vps", bufs=2, space="PSUM") as p1vps:
                hT_all = hTp.tile([128, 4, L], bf16, tag="hTa", name="hTa")
                # split the load by token quarter so the first qkv matmuls
                # start ~2us after kernel begin instead of after the full
                # 2MB transfer
                hT_dv = hT_d[:].rearrange("p (k t) -> p k t", k=4)
                for t4 in range(4):
                    nc.sync.dma_start(
                        out=hT_all[:, :, 512 * t4:512 * (t4 + 1)],
                        in_=hT_dv[:, :, 512 * t4:512 * (t4 + 1)])
                hT_sb = [hT_all[:, kc, :] for kc in range(4)]
                for m in range(HPC):
                    for t4 in range(4):
                        ps = p1ps.tile([128, 512], f32, tag="qkps", name="qkps")
                        for kc in range(4):
                            nc.tensor.matmul(
                                ps[:],
                                wqk_sb[kc][:, 128 * m:128 * (m + 1)],
                                hT_sb[kc][:, 512 * t4:512 * (t4 + 1)],
                                start=(kc == 0), stop=(kc == 3))
                        nc.scalar.copy(
                            out=qk_sb[m][:, 512 * t4:512 * (t4 + 1)], in_=ps[:])
                for c in range(NCH):
                    ps = p1vps.tile([128, 4 * DH], f32, tag="vps", name="vps")
                    for kc in range(4):
                        nc.tensor.matmul(
                            ps[:],
                            hT_sb[kc][:, 128 * c:128 * (c + 1)],
                            wv_sb[kc][:],
                            start=(kc == 0), stop=(kc == 3))
                    pv = ps[:].rearrange("p (h f) -> p h f", f=DH)
                    if c % 2 == 0:
                        nc.scalar.copy(
                            out=v_view[:, c, :, 0:DH], in_=pv[:, :, :])
                    else:
                        nc.vector.tensor_copy(
                            out=v_view[:, c, :, 0:DH], in_=pv[:, :, :])

            # ---- phases 2+3: two heads in flight (parity-tagged tiles) ----
            with tc.tile_pool(name="feat", bufs=1) as featp, \
                 tc.tile_pool(name="misc", bufs=3) as miscp, \
                 tc.tile_pool(name="stsb", bufs=1) as stp, \
                 tc.tile_pool(name="dps", bufs=2, space="PSUM") as dps, \
                 tc.tile_pool(name="bkps", bufs=2, space="PSUM") as bkps, \
                 tc.tile_pool(name="ops", bufs=1, space="PSUM") as ops, \
                 tc.tile_pool(name="stps", bufs=1, space="PSUM") as stps:
                d_del = stps.tile([128, 65 * 4], f32, tag="sdel", name="sdel")
                # qp tiles carry a ones column per chunk (129-col stride)
                qp = {}
                kp = {}
                for p in range(2):
                    qp[p] = [featp.tile([128, NCH * 129], bf16,
                                        tag=f"qp{fc}_{p}", name=f"qp{fc}_{p}")
                             for fc in range(4)]
                    kp[p] = [featp.tile([128, L], bf16, tag=f"kp{fc}_{p}",
                                        name=f"kp{fc}_{p}")
                             for fc in range(4)]
                    for fc in range(4):
                        qv = qp[p][fc][:].rearrange("p (c f) -> p c f", f=129)
                        nc.vector.memset(qv[:, :, 128:129], 1.0)
                for m in range(HPC):
                    p = m % 2
                    # -- prime features, feature-major --
                    # q -> qp (129-stride + ones col), k -> kp (contiguous)
                    for fh in range(2):
                        for t2 in range(2):
                            d_ps = dps.tile([128, 1024], f32, tag="dps",
                                            name="dps")
                            for tt in range(2):
                                t4 = 2 * t2 + tt
                                nc.tensor.matmul(
                                    d_ps[:, 512 * tt:512 * (tt + 1)],
                                    proj_sb[0:64, 128 * fh:128 * (fh + 1)],
                                    qk_sb[m][0:64, 512 * t4:512 * (t4 + 1)],
                                    start=True, stop=True)
                            dv = d_ps[:].rearrange("p (c f) -> p c f", f=128)
                            qv_e = qp[p][fh][:].rearrange(
                                "p (c f) -> p c f", f=129)
                            qv_r = qp[p][fh + 2][:].rearrange(
                                "p (c f) -> p c f", f=129)
                            sl8 = slice(8 * t2, 8 * (t2 + 1))
                            nc.scalar.activation(
                                out=qv_e[:, sl8, 0:128], in_=dv[:, :, :],
                                func=AF.Exp)
                            with nc.allow_low_precision(reason="bf16"):
                                nc.vector.reciprocal(
                                    out=qv_r[:, sl8, 0:128],
                                    in_=qv_e[:, sl8, 0:128])
                    for fh in range(2):
                        for t2 in range(2):
                            d_ps = dps.tile([128, 1024], f32, tag="dps",
                                            name="dps")
                            for tt in range(2):
                                t4 = 2 * t2 + tt
                                nc.tensor.matmul(
                                    d_ps[:, 512 * tt:512 * (tt + 1)],
                                    proj_sb[64:128, 128 * fh:128 * (fh + 1)],
                                    qk_sb[m][64:128, 512 * t4:512 * (t4 + 1)],
                                    start=True, stop=True)
                            sl2 = slice(1024 * t2, 1024 * (t2 + 1))
                            nc.scalar.activation(out=kp[p][fh][:, sl2],
                                                 in_=d_ps[:], func=AF.Exp)
                            # exp(-d): split between ACT (dual-exp) and DVE
                            # (reciprocal) to balance engine load
                            if fh == 0:
                                nc.scalar.activation(
                                    out=kp[p][fh + 2][:, sl2], in_=d_ps[:],
                                    func=AF.Exp, scale=-1.0)
                            else:
                                with nc.allow_low_precision(reason="bf16"):
                                    nc.vector.reciprocal(
                                        out=kp[p][fh + 2][:, sl2],
                                        in_=kp[p][fh][:, sl2])
                    # -- token-major k features (for the state update) --
                    kp_t = featp.tile([128, 512 * NCH], bf16, tag=f"kpt_{p}",
                                      name=f"kpt_{p}")
                    kp_t_v = kp_t[:].rearrange("p (c f) -> p c f", f=512)
                    for qtr in range(4):
                        dt_ps = dps.tile([128, 1024], f32, tag="dps",
                                         name="dtps")
                        dt_v = dt_ps[:].rearrange("p (c f) -> p c f", f=256)
                        for cc in range(4):
                            c = 4 * qtr + cc
                            nc.tensor.matmul(
                                dt_v[:, cc, :],
                                qk_sb[m][64:128, 128 * c:128 * (c + 1)],
                                proj_sb[64:128, :],
                                start=True, stop=True)
                        nc.scalar.activation(
                            out=kp_t_v[:, 4 * qtr:4 * (qtr + 1), 0:256],
                            in_=dt_v[:, :, :], func=AF.Exp)
                        nc.scalar.activation(
                            out=kp_t_v[:, 4 * qtr:4 * (qtr + 1), 256:512],
                            in_=dt_v[:, :, :], func=AF.Exp, scale=-1.0)

                    # -- scan (state in SBUF bf16, updated via delta PSUM) --
                    # per-fc block: cols 0:64 = W state, col 64 = den_acc
                    # state. den col is initialized to EPS_ATTN so the inter
                    # matmul emits denom + eps*qsum in one column.
                    st_sb = stp.tile([128, 65 * 4], bf16, tag=f"st_{p}",
                                     name=f"st_{p}")
                    nc.vector.memset(st_sb, 0.0)
                    for fc in range(4):
                        nc.vector.memset(st_sb[:, 65 * fc + 64:65 * fc + 65],
                                         EPS_ATTN)

                    vp = stp.tile([128, 65], bf16, tag=f"vp_{p}",
                                  name=f"vp_{p}")

                    def emit_bk(c):
                        # keys x queries; col 128 = ksum via qp ones column
                        bkt = bkps.tile([128, 129], f32, tag="bk", name="bk")
                        for fc in range(4):
                            nc.tensor.matmul(
                                bkt[:],
                                kp[p][fc][:, 128 * c:128 * (c + 1)],
                                qp[p][fc][:, 129 * c:129 * c + 129],
                                start=(fc == 0), stop=(fc == 3))
                        return bkt

                    # software-pipeline bk one chunk ahead: bk_{c+1} sits in
                    # the in-order PE queue before delta_c/o_c, so the PE
                    # runs it back-to-back while DVE computes bm_c/rk_c
                    bk = emit_bk(0)
                    for c in range(NCH):
                        bm = miscp.tile([128, 128], bf16, tag=f"bm_{p}",
                                        name=f"bm_{p}")
                        nc.vector.tensor_mul(out=bm[:], in0=bk[:, 0:128],
                                             in1=masku_sb[:])
                        rk = miscp.tile([128, 1], f32, tag=f"rk_{p}",
                                        name=f"rk_{p}")
                        nc.vector.reciprocal(out=rk[:], in_=bk[:, 128:129])
                        if c + 1 < NCH:
                            bk = emit_bk(c + 1)
                        # V' = [v/ksum | 1/ksum]: one op, the ones column
                        # of the 65-wide v block supplies the rk tail
                        nc.gpsimd.tensor_scalar_mul(
                            out=vp[:],
                            in0=v_view[:, c, m, :],
                            scalar1=rk[:])
                        # state delta before o: it only needs vp, so it
                        # fills the PE queue while bm_c is still in flight
                        for fc in range(4):
                            nc.tensor.matmul(
                                d_del[:, 65 * fc:65 * fc + 65],
                                kp_t_v[:, c, 128 * fc:128 * (fc + 1)],
                                vp[:],
                                start=True, stop=True)
                        # out_c = B^T @ V' (intra) + qp_c @ S (inter)
                        o_ps = ops.tile([128, 65], f32, tag="o", name="o")
                        nc.tensor.matmul(o_ps[:], bm[:], vp[:],
                                         start=True, stop=False)
                        for fc in range(4):
                            nc.tensor.matmul(
                                o_ps[:],
                                qp[p][fc][:, 129 * c:129 * c + 128],
                                st_sb[:, 65 * fc:65 * fc + 65],
                                start=False, stop=(fc == 3))
                        # normalize: out / (denom + eps*qsum)
                        rcp = miscp.tile([128, 1], f32, tag=f"rcp_{p}",
                                         name=f"rcp_{p}")
                        nc.vector.reciprocal(out=rcp[:], in_=o_ps[:, 64:65])
                        nc.vector.tensor_scalar_mul(
                            out=on_all[:, 256 * c + 64 * m:
                                       256 * c + 64 * (m + 1)],
                            in0=o_ps[:, 0:64],
                            scalar1=rcp[:])
                        # st += delta (WAR on the o-group's read of st_sb
                        # keeps this ordered after the inter matmuls)
                        with nc.allow_low_precision(reason="bf16 state"):
                            nc.vector.tensor_add(out=st_sb[:], in0=st_sb[:],
                                                 in1=d_del[:])

            # output DMA split by token quarter: each store fires as soon
            # as the last head finishes that quarter of the scan
            for g in range(4):
                sl = slice(1024 * g, 1024 * (g + 1))
                nc.sync.dma_start(out=part_d[:, sl], in_=on_all[:, sl])

    nc.compile()
    return nc


def _host_prep(h, w_qkv, w_o, proj_matrix):
    """Build per-core input maps (bf16, packed)."""
    import ml_dtypes
    bf16 = ml_dtypes.bfloat16

    projs = (proj_matrix * (DH ** -0.25)).astype(np.float32)  # (64, 256)
    masku = (np.arange(128)[:, None] <= np.arange(128)[None, :])

    in_maps = []
    for core in range(N_CORES):
        b, hg = core // 2, core % 2
        heads = [HPC * hg + mm for mm in range(HPC)]
        # hT packed [p, kc, t]: original row = 128*kc + p
        hT = h[:, b, :].T.reshape(4, 128, L).transpose(1, 0, 2).reshape(
            128, 4 * L)
        wqkT = np.empty((DM, 128 * HPC), np.float32)
        wvT = np.empty((DM, 64 * HPC), np.float32)
        for mm, hh in enumerate(heads):
            blk = w_qkv[192 * hh:192 * (hh + 1)]  # (192, DM) = [q64,k64,v64]
            wqkT[:, 128 * mm:128 * mm + 64] = blk[0:64].T
            wqkT[:, 128 * mm + 64:128 * (mm + 1)] = blk[64:128].T
            wvT[:, 64 * mm:64 * (mm + 1)] = blk[128:192].T
        blob = np.zeros((128, BLOB_COLS), np.float32)
        blob[:, BLOB_WQK:BLOB_WQK + 2048] = wqkT.reshape(
            4, 128, 512).transpose(1, 0, 2).reshape(128, 2048)
        blob[:, BLOB_WV:BLOB_WV + 1024] = wvT.reshape(
            4, 128, 256).transpose(1, 0, 2).reshape(128, 1024)
        blob[:, BLOB_PROJ:BLOB_PROJ + 256] = np.concatenate([projs, projs], 0)
        blob[:, BLOB_MASK:BLOB_MASK + 128] = masku
        in_maps.append({
            "hT": hT.astype(bf16),
            "blob": blob.astype(bf16),
        })
    return in_maps


def kernel(h, w_qkv, w_o, ln_gamma, ln_beta, proj_matrix):
    from concourse.bass_utils import run_bass_kernel_spmd

    h = np.asarray(h, np.float32)
    w_qkv = np.asarray(w_qkv, np.float32)
    w_o = np.asarray(w_o, np.float32)
    ln_gamma = np.asarray(ln_gamma, np.float32)
    ln_beta = np.asarray(ln_beta, np.float32)
    proj_matrix = np.asarray(proj_matrix, np.float32)

    if "nc" not in _CACHE:
        _CACHE["nc"] = _build_nc()
    nc = _CACHE["nc"]

    in_maps = _host_prep(h, w_qkv, w_o, proj_matrix)
    res = run_bass_kernel_spmd(nc, in_maps, core_ids=list(range(N_CORES)))

    woT = (w_o.T * SCALE).astype(np.float32)  # (H*DH, DM)
    out = np.empty((L, B, DM), np.float32)
    for b in range(B):
        halves = []
        for hg in range(2):
            raw = np.asarray(res.results[2 * b + hg]["part"])
            on = raw.astype(np.float32).reshape(128, NCH, 4, DH)
            halves.append(on.transpose(1, 0, 2, 3).reshape(L, 4 * DH))
        attn = np.concatenate(halves, axis=1) @ woT  # (L, DM)
        x = h[:, b, :] + attn
        mu = x.mean(-1, keepdims=True)
        var = ((x - mu) ** 2).mean(-1, keepdims=True)
        out[:, b, :] = (x - mu) / np.sqrt(var + EPS_LN) * ln_gamma + ln_beta
    return out


# revision 31
# speedup vs baseline: 1.0233x; 1.0233x over previous
"""Trainium2 Bass kernel for nn_CudaFastWeightSumPerformerLayer.

Performer FAVOR+ fast-weight (causal linear attention) layer.
Sharding: 8 cores = 4 batches x 2 head-groups (4 heads each). Each core
computes qkv projection, prime features, and the chunked causal
linear-attention scan in bf16, emitting the normalized per-head attention
output (B,heads,L,dh) as one bf16 tensor. Host applies the (small) w_o
projection, residual, and LayerNorm in f32.

Math restructure (validated vs reference):
  - The FAVOR+ diag term exp(-0.5|x|^2) cancels in the normalized output,
    so features are just [exp(d), exp(-d)], d = (x * dh^-0.25) @ proj.
  - kp normalization (1/sum) is folded into V' columns; the attention
    denominator and the q-feature sum arrive as extra output columns via
    ones-columns in V' and the scan state.
  - ksum (per-key feature sum) is obtained for free as an extra column of
    the B matrix by appending a ones column to the qp chunk (129-col rhs).
  - out_final = out_raw / (denom_raw + eps * qsum).
Chunked scan (chunk=128): B[j,t] = kp_j . qp_t (masked j<=t),
  out_c = B^T @ V' + qp_c @ S;  S += kp_c^T @ V'.
All matmuls bf16 with f32 PSUM accumulation. I/O is bf16 and packed into
two input DMAs and one output DMA, each contiguous per partition.
"""

import numpy as np

L, DM, DH, M = 2048, 512, 64, 256
F = 2 * M          # 512 feature dim
NH = 8             # total heads
HPC = 4            # heads per core
B = 4
CH = 128           # scan chunk
NCH = L // CH      # 16
SCALE = DH ** -0.5
EPS_ATTN = 1e-5
EPS_LN = 1e-5
N_CORES = 8

# packed weight blob column offsets (bf16, per partition)
BLOB_WQK = 0            # [p, kc, 512]   kc in 0..3   (2048 cols)
BLOB_WV = 2048          # [p, kc, 256]                (1024 cols)
BLOB_PROJ = 3072        # [p, 256] proj duplicated on both 64-halves
BLOB_MASK = 3328        # [p, 128] upper-triangular mask (j<=t)
BLOB_COLS = 3456

_CACHE = {}


def _build_nc():
    import concourse.bacc as bacc
    import concourse.tile as tile
    from concourse import mybir

    f32 = mybir.dt.float32
    bf16 = mybir.dt.bfloat16
    AF = mybir.ActivationFunctionType
    ALU = mybir.AluOpType

    nc = bacc.Bacc("TRN2", target_bir_lowering=False, debug=False,
                   num_devices=N_CORES)

    hT_d = nc.dram_tensor("hT", [128, 4 * L], bf16, kind="ExternalInput")
    blob_d = nc.dram_tensor("blob", [128, BLOB_COLS], bf16,
                            kind="ExternalInput")
    part_d = nc.dram_tensor("part", [128, NCH * 4 * DH], bf16,
                            kind="ExternalOutput")

    with tile.TileContext(nc) as tc:
        from contextlib import ExitStack
        with ExitStack() as ctx:
            consts = ctx.enter_context(tc.tile_pool(name="consts", bufs=1))
            qkpool = ctx.enter_context(tc.tile_pool(name="qkpool", bufs=1))
            vpool = ctx.enter_context(tc.tile_pool(name="vpool", bufs=1))
            onorm = ctx.enter_context(tc.tile_pool(name="onorm", bufs=1))

            blob = consts.tile([128, BLOB_COLS], bf16, tag="blob",
                               name="blob")
            nc.sync.dma_start(out=blob, in_=blob_d[:, :])
            wqk_sb = [blob[:, BLOB_WQK + 512 * kc:BLOB_WQK + 512 * (kc + 1)]
                      for kc in range(4)]
            wv_sb = [blob[:, BLOB_WV + 256 * kc:BLOB_WV + 256 * (kc + 1)]
                     for kc in range(4)]
            proj_sb = blob[:, BLOB_PROJ:BLOB_PROJ + 256]
            masku_sb = blob[:, BLOB_MASK:BLOB_MASK + 128]

            # scan output, token-major: [p, chunk, 4 heads x 64] bf16
            on_all = onorm.tile([128, NCH * 4 * DH], bf16, tag="on",
                                name="on")

            # ---- phase 1: qkv projection (bf16) ----
            qk_sb = [qkpool.tile([128, L], bf16, tag=f"qk{m}", name=f"qk{m}")
                     for m in range(HPC)]
            # v, token-major: [p, chunk, 4 heads x 65] bf16; col 64 of each
            # head block is preset to 1.0 so V' = v_block * rk in one op
            v_all = vpool.tile([128, NCH * 4 * (DH + 1)], bf16, tag="v",
                               name="v")
            v_view = v_all[:].rearrange("p (c h f) -> p c h f", h=4, f=DH + 1)
            for mm in range(4):
                nc.vector.memset(v_view[:, :, mm, 64:65], 1.0)
            with tc.tile_pool(name="hTp", bufs=1) as hTp, \
                 tc.tile_pool(name="p1ps", bufs=2, space="PSUM") as p1ps, \
                 tc.tile_pool(name="p1vps", bufs=2, space="PSUM") as p1vps:
                hT_all = hTp.tile([128, 4, L], bf16, tag="hTa", name="hTa")
                # split the load by token quarter so the first qkv matmuls
                # start ~2us after kernel begin instead of after the full
                # 2MB transfer
                hT_dv = hT_d[:].rearrange("p (k t) -> p k t", k=4)
                for t4 in range(4):
                    nc.sync.dma_start(
                        out=hT_all[:, :, 512 * t4:512 * (t4 + 1)],
                        in_=hT_dv[:, :, 512 * t4:512 * (t4 + 1)])
                hT_sb = [hT_all[:, kc, :] for kc in range(4)]
                for m in range(HPC):
                    for t4 in range(4):
                        ps = p1ps.tile([128, 512], f32, tag="qkps", name="qkps")
                        for kc in range(4):
                            nc.tensor.matmul(
                                ps[:],
                                wqk_sb[kc][:, 128 * m:128 * (m + 1)],
                                hT_sb[kc][:, 512 * t4:512 * (t4 + 1)],
                                start=(kc == 0), stop=(kc == 3))
                        nc.scalar.copy(
                            out=qk_sb[m][:, 512 * t4:512 * (t4 + 1)], in_=ps[:])
                for c in range(NCH):
                    ps = p1vps.tile([128, 4 * DH], f32, tag="vps", name="vps")
                    for kc in range(4):
                        nc.tensor.matmul(
                            ps[:],
                            hT_sb[kc][:, 128 * c:128 * (c + 1)],
                            wv_sb[kc][:],
                            start=(kc == 0), stop=(kc == 3))
                    pv = ps[:].rearrange("p (h f) -> p h f", f=DH)
                    if c % 2 == 0:
                        nc.scalar.copy(
                            out=v_view[:, c, :, 0:DH], in_=pv[:, :, :])
                    else:
                        nc.vector.tensor_copy(
                            out=v_view[:, c, :, 0:DH], in_=pv[:, :, :])

            # ---- phases 2+3: two heads in flight (parity-tagged tiles) ----
            with tc.tile_pool(name="feat", bufs=1) as featp, \
                 tc.tile_pool(name="misc", bufs=3) as miscp, \
                 tc.tile_pool(name="stsb", bufs=2) as stp, \
                 tc.tile_pool(name="dps", bufs=2, space="PSUM") as dps, \
                 tc.tile_pool(name="bkps", bufs=2, space="PSUM") as bkps, \
                 tc.tile_pool(name="ops", bufs=1, space="PSUM") as ops, \
                 tc.tile_pool(name="stps", bufs=1, space="PSUM") as stps:
                d_del = stps.tile([128, 65 * 4], f32, tag="sdel", name="sdel")
                # qp tiles carry a ones column per chunk (129-col stride)
                qp = {}
                kp = {}
                for p in range(2):
                    qp[p] = [featp.tile([128, NCH * 129], bf16,
                                        tag=f"qp{fc}_{p}", name=f"qp{fc}_{p}")
                             for fc in range(4)]
                    kp[p] = [featp.tile([128, L], bf16, tag=f"kp{fc}_{p}",
                                        name=f"kp{fc}_{p}")
                             for fc in range(4)]
                    for fc in range(4):
                        qv = qp[p][fc][:].rearrange("p (c f) -> p c f", f=129)
                        nc.vector.memset(qv[:, :, 128:129], 1.0)
                for m in range(HPC):
                    p = m % 2
                    # -- prime features, feature-major --
                    # q -> qp (129-stride + ones col), k -> kp (contiguous)
                    for fh in range(2):
                        for t2 in range(2):
                            d_ps = dps.tile([128, 1024], f32, tag="dps",
                                            name="dps")
                            for tt in range(2):
                                t4 = 2 * t2 + tt
                                nc.tensor.matmul(
                                    d_ps[:, 512 * tt:512 * (tt + 1)],
                                    proj_sb[0:64, 128 * fh:128 * (fh + 1)],
                                    qk_sb[m][0:64, 512 * t4:512 * (t4 + 1)],
                                    start=True, stop=True)
                            dv = d_ps[:].rearrange("p (c f) -> p c f", f=128)
                            qv_e = qp[p][fh][:].rearrange(
                                "p (c f) -> p c f", f=129)
                            qv_r = qp[p][fh + 2][:].rearrange(
                                "p (c f) -> p c f", f=129)
                            sl8 = slice(8 * t2, 8 * (t2 + 1))
                            nc.scalar.activation(
                                out=qv_e[:, sl8, 0:128], in_=dv[:, :, :],
                                func=AF.Exp)
                            with nc.allow_low_precision(reason="bf16"):
                                nc.vector.reciprocal(
                                    out=qv_r[:, sl8, 0:128],
                                    in_=qv_e[:, sl8, 0:128])
                    for fh in range(2):
                        for t2 in range(2):
                            d_ps = dps.tile([128, 1024], f32, tag="dps",
                                            name="dps")
                            for tt in range(2):
                                t4 = 2 * t2 + tt
                                nc.tensor.matmul(
                                    d_ps[:, 512 * tt:512 * (tt + 1)],
                                    proj_sb[64:128, 128 * fh:128 * (fh + 1)],
                                    qk_sb[m][64:128, 512 * t4:512 * (t4 + 1)],
                                    start=True, stop=True)
                            sl2 = slice(1024 * t2, 1024 * (t2 + 1))
                            nc.scalar.activation(out=kp[p][fh][:, sl2],
                                                 in_=d_ps[:], func=AF.Exp)
                            # exp(-d): split between ACT (dual-exp) and DVE
                            # (reciprocal) to balance engine load
                            if fh == 0:
                                nc.scalar.activation(
                                    out=kp[p][fh + 2][:, sl2], in_=d_ps[:],
                                    func=AF.Exp, scale=-1.0)
                            else:
                                with nc.allow_low_precision(reason="bf16"):
                                    nc.vector.reciprocal(
                                        out=kp[p][fh + 2][:, sl2],
                                        in_=kp[p][fh][:, sl2])
                    # -- token-major k features (for the state update) --
                    kp_t = featp.tile([128, 512 * NCH], bf16, tag=f"kpt_{p}",
                                      name=f"kpt_{p}")
                    kp_t_v = kp_t[:].rearrange("p (c f) -> p c f", f=512)
                    for qtr in range(4):
                        dt_ps = dps.tile([128, 1024], f32, tag="dps",
                                         name="dtps")
                        dt_v = dt_ps[:].rearrange("p (c f) -> p c f", f=256)
                        for cc in range(4):
                            c = 4 * qtr + cc
                            nc.tensor.matmul(
                                dt_v[:, cc, :],
                                qk_sb[m][64:128, 128 * c:128 * (c + 1)],
                                proj_sb[64:128, :],
                                start=True, stop=True)
                        nc.scalar.activation(
                            out=kp_t_v[:, 4 * qtr:4 * (qtr + 1), 0:256],
                            in_=dt_v[:, :, :], func=AF.Exp)
                        nc.scalar.activation(
                            out=kp_t_v[:, 4 * qtr:4 * (qtr + 1), 256:512],
                            in_=dt_v[:, :, :], func=AF.Exp, scale=-1.0)

                    # -- scan (state in SBUF bf16, ping-ponged so the
                    # out-of-place add never write-blocks on the o-group's
                    # read of the previous state). per-fc block: cols 0:64 =
                    # W state, col 64 = den_acc state (init EPS_ATTN so the
                    # inter matmul emits denom + eps*qsum in one column).
                    st_cur = stp.tile([128, 65 * 4], bf16, tag=f"st_{p}",
                                      name=f"st_{p}")
                    nc.vector.memset(st_cur, 0.0)
                    for fc in range(4):
                        nc.vector.memset(st_cur[:, 65 * fc + 64:65 * fc + 65],
                                         EPS_ATTN)

                    vp = stp.tile([128, 65], bf16, tag=f"vp_{p}",
                                  name=f"vp_{p}")

                    def emit_bk(c):
                        # keys x queries; col 128 = ksum via qp ones column
                        bkt = bkps.tile([128, 129], f32, tag="bk", name="bk")
                        for fc in range(4):
                            nc.tensor.matmul(
                                bkt[:],
                                kp[p][fc][:, 128 * c:128 * (c + 1)],
                                qp[p][fc][:, 129 * c:129 * c + 129],
                                start=(fc == 0), stop=(fc == 3))
                        return bkt

                    # software-pipeline bk one chunk ahead: bk_{c+1} sits in
                    # the in-order PE queue before delta_c/o_c, so the PE
                    # runs it back-to-back while DVE computes bm_c/rk_c
                    bk = emit_bk(0)
                    for c in range(NCH):
                        bm = miscp.tile([128, 128], bf16, tag=f"bm_{p}",
                                        name=f"bm_{p}")
                        nc.vector.tensor_mul(out=bm[:], in0=bk[:, 0:128],
                                             in1=masku_sb[:])
                        rk = miscp.tile([128, 1], f32, tag=f"rk_{p}",
                                        name=f"rk_{p}")
                        nc.vector.reciprocal(out=rk[:], in_=bk[:, 128:129])
                        if c + 1 < NCH:
                            bk = emit_bk(c + 1)
                        # V' = [v/ksum | 1/ksum]: one op, the ones column
                        # of the 65-wide v block supplies the rk tail
                        nc.gpsimd.tensor_scalar_mul(
                            out=vp[:],
                            in0=v_view[:, c, m, :],
                            scalar1=rk[:])
                        # state delta before o: it only needs vp, so it
                        # fills the PE queue while bm_c is still in flight
                        for fc in range(4):
                            nc.tensor.matmul(
                                d_del[:, 65 * fc:65 * fc + 65],
                                kp_t_v[:, c, 128 * fc:128 * (fc + 1)],
                                vp[:],
                                start=True, stop=True)
                        # out_c = B^T @ V' (intra) + qp_c @ S (inter)
                        o_ps = ops.tile([128, 65], f32, tag="o", name="o")
                        nc.tensor.matmul(o_ps[:], bm[:], vp[:],
                                         start=True, stop=False)
                        for fc in range(4):
                            nc.tensor.matmul(
                                o_ps[:],
                                qp[p][fc][:, 129 * c:129 * c + 128],
                                st_cur[:, 65 * fc:65 * fc + 65],
                                start=False, stop=(fc == 3))
                        # normalize: out / (denom + eps*qsum)
                        rcp = miscp.tile([128, 1], f32, tag=f"rcp_{p}",
                                         name=f"rcp_{p}")
                        nc.vector.reciprocal(out=rcp[:], in_=o_ps[:, 64:65])
                        nc.vector.tensor_scalar_mul(
                            out=on_all[:, 256 * c + 64 * m:
                                       256 * c + 64 * (m + 1)],
                            in0=o_ps[:, 0:64],
                            scalar1=rcp[:])
                        # out-of-place state add into the other ping-pong
                        # buffer: only a read of st_cur, so it runs as soon
                        # as delta lands
                        st_new = stp.tile([128, 65 * 4], bf16, tag=f"st_{p}",
                                          name=f"st_{p}")
                        with nc.allow_low_precision(reason="bf16 state"):
                            nc.vector.tensor_add(out=st_new[:], in0=st_cur[:],
                                                 in1=d_del[:])
                        st_cur = st_new

            # output DMA split by token quarter: each store fires as soon
            # as the last head finishes that quarter of the scan
            for g in range(4):
                sl = slice(1024 * g, 1024 * (g + 1))
                nc.sync.dma_start(out=part_d[:, sl], in_=on_all[:, sl])

    nc.compile()
    return nc


def _host_prep(h, w_qkv, w_o, proj_matrix):
    """Build per-core input maps (bf16, packed)."""
    import ml_dtypes
    bf16 = ml_dtypes.bfloat16

    projs = (proj_matrix * (DH ** -0.25)).astype(np.float32)  # (64, 256)
    masku = (np.arange(128)[:, None] <= np.arange(128)[None, :])

    in_maps = []
    for core in range(N_CORES):
        b, hg = core // 2, core % 2
        heads = [HPC * hg + mm for mm in range(HPC)]
        # hT packed [p, kc, t]: original row = 128*kc + p
        hT = h[:, b, :].T.reshape(4, 128, L).transpose(1, 0, 2).reshape(
            128, 4 * L)
        wqkT = np.empty((DM, 128 * HPC), np.float32)
        wvT = np.empty((DM, 64 * HPC), np.float32)
        for mm, hh in enumerate(heads):
            blk = w_qkv[192 * hh:192 * (hh + 1)]  # (192, DM) = [q64,k64,v64]
            wqkT[:, 128 * mm:128 * mm + 64] = blk[0:64].T
            wqkT[:, 128 * mm + 64:128 * (mm + 1)] = blk[64:128].T
            wvT[:, 64 * mm:64 * (mm + 1)] = blk[128:192].T
        blob = np.zeros((128, BLOB_COLS), np.float32)
        blob[:, BLOB_WQK:BLOB_WQK + 2048] = wqkT.reshape(
            4, 128, 512).transpose(1, 0, 2).reshape(128, 2048)
        blob[:, BLOB_WV:BLOB_WV + 1024] = wvT.reshape(
            4, 128, 256).transpose(1, 0, 2).reshape(128, 1024)
        blob[:, BLOB_PROJ:BLOB_PROJ + 256] = np.concatenate([projs, projs], 0)
        blob[:, BLOB_MASK:BLOB_MASK + 128] = masku
        in_maps.append({
            "hT": hT.astype(bf16),
            "blob": blob.astype(bf16),
        })
    return in_maps


def kernel(h, w_qkv, w_o, ln_gamma, ln_beta, proj_matrix):
    from concourse.bass_utils import run_bass_kernel_spmd

    h = np.asarray(h, np.float32)
    w_qkv = np.asarray(w_qkv, np.float32)
    w_o = np.asarray(w_o, np.float32)
    ln_gamma = np.asarray(ln_gamma, np.float32)
    ln_beta = np.asarray(ln_beta, np.float32)
    proj_matrix = np.asarray(proj_matrix, np.float32)

    if "nc" not in _CACHE:
        _CACHE["nc"] = _build_nc()
    nc = _CACHE["nc"]

    in_maps = _host_prep(h, w_qkv, w_o, proj_matrix)
    res = run_bass_kernel_spmd(nc, in_maps, core_ids=list(range(N_CORES)))

    woT = (w_o.T * SCALE).astype(np.float32)  # (H*DH, DM)
    out = np.empty((L, B, DM), np.float32)
    for b in range(B):
        halves = []
        for hg in range(2):
            raw = np.asarray(res.results[2 * b + hg]["part"])
            on = raw.astype(np.float32).reshape(128, NCH, 4, DH)
            halves.append(on.transpose(1, 0, 2, 3).reshape(L, 4 * DH))
        attn = np.concatenate(halves, axis=1) @ woT  # (L, DM)
        x = h[:, b, :] + attn
        mu = x.mean(-1, keepdims=True)
        var = ((x - mu) ** 2).mean(-1, keepdims=True)
        out[:, b, :] = (x - mu) / np.sqrt(var + EPS_LN) * ln_gamma + ln_beta
    return out


# revision 36
# speedup vs baseline: 1.1317x; 1.1059x over previous
"""Trainium2 Bass kernel for nn_CudaFastWeightSumPerformerLayer.

Performer FAVOR+ fast-weight (causal linear attention) layer.
Sharding: 8 cores = 4 batches x 2 head-groups (4 heads each). Each core
computes qkv projection, prime features, and the chunked causal
linear-attention scan in bf16, emitting the normalized per-head attention
output (B,heads,L,dh) as one bf16 tensor. Host applies the (small) w_o
projection, residual, and LayerNorm in f32.

Math restructure (validated vs reference):
  - The FAVOR+ diag term exp(-0.5|x|^2) cancels in the normalized output,
    so features are just [exp(d), exp(-d)], d = (x * dh^-0.25) @ proj.
  - kp normalization (1/sum) is folded into V' columns; the attention
    denominator and the q-feature sum arrive as extra output columns via
    ones-columns in V' and the scan state.
  - ksum (per-key feature sum) is obtained for free as an extra column of
    the B matrix by appending a ones column to the qp chunk (129-col rhs).
  - out_final = out_raw / (denom_raw + eps * qsum).
Chunked scan (chunk=128): B[j,t] = kp_j . qp_t (masked j<=t),
  out_c = B^T @ V' + qp_c @ S;  S += kp_c^T @ V'.
All matmuls bf16 with f32 PSUM accumulation. I/O is bf16 and packed into
two input DMAs and one output DMA, each contiguous per partition.
"""

import numpy as np

L, DM, DH, M = 2048, 512, 64, 256
F = 2 * M          # 512 feature dim
NH = 8             # total heads
HPC = 4            # heads per core
B = 4
CH = 128           # scan chunk
NCH = L // CH      # 16
SCALE = DH ** -0.5
EPS_ATTN = 1e-5
EPS_LN = 1e-5
N_CORES = 8

# packed weight blob column offsets (bf16, per partition)
BLOB_WQK = 0            # [p, kc, 512]   kc in 0..3   (2048 cols)
BLOB_WV = 2048          # [p, kc, 256]                (1024 cols)
BLOB_PROJ = 3072        # [p, 256] proj duplicated on both 64-halves
BLOB_MASK = 3328        # [p, 128] upper-triangular mask (j<=t)
BLOB_COLS = 3456

_CACHE = {}


def _build_nc():
    import concourse.bacc as bacc
    import concourse.tile as tile
    from concourse import mybir

    f32 = mybir.dt.float32
    bf16 = mybir.dt.bfloat16
    AF = mybir.ActivationFunctionType
    ALU = mybir.AluOpType

    nc = bacc.Bacc("TRN2", target_bir_lowering=False, debug=False,
                   num_devices=N_CORES)

    hT_d = nc.dram_tensor("hT", [128, 4 * L], bf16, kind="ExternalInput")
    blob_d = nc.dram_tensor("blob", [128, BLOB_COLS], bf16,
                            kind="ExternalInput")
    part_d = nc.dram_tensor("part", [128, NCH * 4 * DH], bf16,
                            kind="ExternalOutput")

    with tile.TileContext(nc) as tc:
        from contextlib import ExitStack
        with ExitStack() as ctx:
            consts = ctx.enter_context(tc.tile_pool(name="consts", bufs=1))
            qkpool = ctx.enter_context(tc.tile_pool(name="qkpool", bufs=1))
            vpool = ctx.enter_context(tc.tile_pool(name="vpool", bufs=1))
            onorm = ctx.enter_context(tc.tile_pool(name="onorm", bufs=1))

            blob = consts.tile([128, BLOB_COLS], bf16, tag="blob",
                               name="blob")
            nc.sync.dma_start(out=blob, in_=blob_d[:, :])
            wqk_sb = [blob[:, BLOB_WQK + 512 * kc:BLOB_WQK + 512 * (kc + 1)]
                      for kc in range(4)]
            wv_sb = [blob[:, BLOB_WV + 256 * kc:BLOB_WV + 256 * (kc + 1)]
                     for kc in range(4)]
            proj_sb = blob[:, BLOB_PROJ:BLOB_PROJ + 256]
            masku_sb = blob[:, BLOB_MASK:BLOB_MASK + 128]

            # scan output, token-major: [p, chunk, 4 heads x 64] bf16
            on_all = onorm.tile([128, NCH * 4 * DH], bf16, tag="on",
                                name="on")

            # ---- phase 1: qkv projection (bf16) ----
            qk_sb = [qkpool.tile([128, L], bf16, tag=f"qk{m}", name=f"qk{m}")
                     for m in range(HPC)]
            # v, token-major: [p, chunk, 4 heads x 65] bf16; col 64 of each
            # head block is preset to 1.0 so V' = v_block * rk in one op
            v_all = vpool.tile([128, NCH * 4 * (DH + 1)], bf16, tag="v",
                               name="v")
            v_view = v_all[:].rearrange("p (c h f) -> p c h f", h=4, f=DH + 1)
            for mm in range(4):
                nc.vector.memset(v_view[:, :, mm, 64:65], 1.0)
            with tc.tile_pool(name="hTp", bufs=1) as hTp, \
                 tc.tile_pool(name="p1ps", bufs=2, space="PSUM") as p1ps, \
                 tc.tile_pool(name="p1vps", bufs=2, space="PSUM") as p1vps:
                hT_all = hTp.tile([128, 4, L], bf16, tag="hTa", name="hTa")
                # split the load by token quarter so the first qkv matmuls
                # start ~2us after kernel begin instead of after the full
                # 2MB transfer
                hT_dv = hT_d[:].rearrange("p (k t) -> p k t", k=4)
                for t4 in range(4):
                    nc.sync.dma_start(
                        out=hT_all[:, :, 512 * t4:512 * (t4 + 1)],
                        in_=hT_dv[:, :, 512 * t4:512 * (t4 + 1)])
                hT_sb = [hT_all[:, kc, :] for kc in range(4)]
                for m in range(HPC):
                    for t4 in range(4):
                        ps = p1ps.tile([128, 512], f32, tag="qkps", name="qkps")
                        for kc in range(4):
                            nc.tensor.matmul(
                                ps[:],
                                wqk_sb[kc][:, 128 * m:128 * (m + 1)],
                                hT_sb[kc][:, 512 * t4:512 * (t4 + 1)],
                                start=(kc == 0), stop=(kc == 3))
                        nc.scalar.copy(
                            out=qk_sb[m][:, 512 * t4:512 * (t4 + 1)], in_=ps[:])
                for c in range(NCH):
                    ps = p1vps.tile([128, 4 * DH], f32, tag="vps", name="vps")
                    for kc in range(4):
                        nc.tensor.matmul(
                            ps[:],
                            hT_sb[kc][:, 128 * c:128 * (c + 1)],
                            wv_sb[kc][:],
                            start=(kc == 0), stop=(kc == 3))
                    pv = ps[:].rearrange("p (h f) -> p h f", f=DH)
                    if c % 2 == 0:
                        nc.scalar.copy(
                            out=v_view[:, c, :, 0:DH], in_=pv[:, :, :])
                    else:
                        nc.vector.tensor_copy(
                            out=v_view[:, c, :, 0:DH], in_=pv[:, :, :])

            # ---- phases 2+3: two heads in flight (parity-tagged tiles) ----
            with tc.tile_pool(name="feat", bufs=1) as featp, \
                 tc.tile_pool(name="misc", bufs=3) as miscp, \
                 tc.tile_pool(name="stsb", bufs=2) as stp, \
                 tc.tile_pool(name="dps", bufs=2, space="PSUM") as dps, \
                 tc.tile_pool(name="bkps", bufs=2, space="PSUM") as bkps, \
                 tc.tile_pool(name="ops", bufs=1, space="PSUM") as ops, \
                 tc.tile_pool(name="stps", bufs=1, space="PSUM") as stps:
                d_del = stps.tile([128, 65 * 4], f32, tag="sdel", name="sdel")
                # qp tiles carry a ones column per chunk (129-col stride)
                qp = {}
                kp = {}
                for p in range(2):
                    qp[p] = [featp.tile([128, NCH * 129], bf16,
                                        tag=f"qp{fc}_{p}", name=f"qp{fc}_{p}")
                             for fc in range(4)]
                    kp[p] = [featp.tile([128, L], bf16, tag=f"kp{fc}_{p}",
                                        name=f"kp{fc}_{p}")
                             for fc in range(4)]
                    for fc in range(4):
                        qv = qp[p][fc][:].rearrange("p (c f) -> p c f", f=129)
                        nc.vector.memset(qv[:, :, 128:129], 1.0)
                for m in range(HPC):
                    p = m % 2
                    # -- prime features, feature-major --
                    # q -> qp (129-stride + ones col), k -> kp (contiguous)
                    for fh in range(2):
                        for t2 in range(2):
                            d_ps = dps.tile([128, 1024], f32, tag="dps",
                                            name="dps")
                            for tt in range(2):
                                t4 = 2 * t2 + tt
                                nc.tensor.matmul(
                                    d_ps[:, 512 * tt:512 * (tt + 1)],
                                    proj_sb[0:64, 128 * fh:128 * (fh + 1)],
                                    qk_sb[m][0:64, 512 * t4:512 * (t4 + 1)],
                                    start=True, stop=True)
                            dv = d_ps[:].rearrange("p (c f) -> p c f", f=128)
                            qv_e = qp[p][fh][:].rearrange(
                                "p (c f) -> p c f", f=129)
                            qv_r = qp[p][fh + 2][:].rearrange(
                                "p (c f) -> p c f", f=129)
                            sl8 = slice(8 * t2, 8 * (t2 + 1))
                            nc.scalar.activation(
                                out=qv_e[:, sl8, 0:128], in_=dv[:, :, :],
                                func=AF.Exp)
                            with nc.allow_low_precision(reason="bf16"):
                                nc.vector.reciprocal(
                                    out=qv_r[:, sl8, 0:128],
                                    in_=qv_e[:, sl8, 0:128])
                    for fh in range(2):
                        for t2 in range(2):
                            d_ps = dps.tile([128, 1024], f32, tag="dps",
                                            name="dps")
                            for tt in range(2):
                                t4 = 2 * t2 + tt
                                nc.tensor.matmul(
                                    d_ps[:, 512 * tt:512 * (tt + 1)],
                                    proj_sb[64:128, 128 * fh:128 * (fh + 1)],
                                    qk_sb[m][64:128, 512 * t4:512 * (t4 + 1)],
                                    start=True, stop=True)
                            sl2 = slice(1024 * t2, 1024 * (t2 + 1))
                            nc.scalar.activation(out=kp[p][fh][:, sl2],
                                                 in_=d_ps[:], func=AF.Exp)
                            # exp(-d): split between ACT (dual-exp) and DVE
                            # (reciprocal) to balance engine load
                            if fh == 0:
                                nc.scalar.activation(
                                    out=kp[p][fh + 2][:, sl2], in_=d_ps[:],
                                    func=AF.Exp, scale=-1.0)
                            else:
                                with nc.allow_low_precision(reason="bf16"):
                                    nc.vector.reciprocal(
                                        out=kp[p][fh + 2][:, sl2],
                                        in_=kp[p][fh][:, sl2])
                    # -- token-major k features (for the state update) --
                    kp_t = featp.tile([128, 512 * NCH], bf16, tag=f"kpt_{p}",
                                      name=f"kpt_{p}")
                    kp_t_v = kp_t[:].rearrange("p (c f) -> p c f", f=512)
                    for qtr in range(4):
                        dt_ps = dps.tile([128, 1024], f32, tag="dps",
                                         name="dtps")
                        dt_v = dt_ps[:].rearrange("p (c f) -> p c f", f=256)
                        for cc in range(4):
                            c = 4 * qtr + cc
                            nc.tensor.matmul(
                                dt_v[:, cc, :],
                                qk_sb[m][64:128, 128 * c:128 * (c + 1)],
                                proj_sb[64:128, :],
                                start=True, stop=True)
                        nc.scalar.activation(
                            out=kp_t_v[:, 4 * qtr:4 * (qtr + 1), 0:256],
                            in_=dt_v[:, :, :], func=AF.Exp)
                        nc.scalar.activation(
                            out=kp_t_v[:, 4 * qtr:4 * (qtr + 1), 256:512],
                            in_=dt_v[:, :, :], func=AF.Exp, scale=-1.0)

                    # -- scan (state in SBUF bf16, ping-ponged so the
                    # out-of-place add never write-blocks on the o-group's
                    # read of the previous state). per-fc block: cols 0:64 =
                    # W state, col 64 = den_acc state (init EPS_ATTN so the
                    # inter matmul emits denom + eps*qsum in one column).
                    st_cur = stp.tile([128, 65 * 4], bf16, tag=f"st_{p}",
                                      name=f"st_{p}")
                    nc.vector.memset(st_cur, 0.0)
                    for fc in range(4):
                        nc.vector.memset(st_cur[:, 65 * fc + 64:65 * fc + 65],
                                         EPS_ATTN)

                    vp = stp.tile([128, 65], bf16, tag=f"vp_{p}",
                                  name=f"vp_{p}")

                    def emit_bk(c):
                        # keys x queries; col 128 = ksum via qp ones column
                        bkt = bkps.tile([128, 129], f32, tag="bk", name="bk")
                        for fc in range(4):
                            nc.tensor.matmul(
                                bkt[:],
                                kp[p][fc][:, 128 * c:128 * (c + 1)],
                                qp[p][fc][:, 129 * c:129 * c + 129],
                                start=(fc == 0), stop=(fc == 3))
                        return bkt

                    # software-pipeline bk one chunk ahead: bk_{c+1} sits in
                    # the in-order PE queue before delta_c/o_c, so the PE
                    # runs it back-to-back while DVE computes bm_c/rk_c
                    bk = emit_bk(0)
                    for c in range(NCH):
                        bm = miscp.tile([128, 128], bf16, tag=f"bm_{p}",
                                        name=f"bm_{p}")
                        nc.vector.tensor_mul(out=bm[:], in0=bk[:, 0:128],
                                             in1=masku_sb[:])
                        rk = miscp.tile([128, 1], f32, tag=f"rk_{p}",
                                        name=f"rk_{p}")
                        nc.vector.reciprocal(out=rk[:], in_=bk[:, 128:129])
                        if c + 1 < NCH:
                            bk = emit_bk(c + 1)
                        # V' = [v/ksum | 1/ksum]: one op, the ones column
                        # of the 65-wide v block supplies the rk tail
                        nc.gpsimd.tensor_scalar_mul(
                            out=vp[:],
                            in0=v_view[:, c, m, :],
                            scalar1=rk[:])
                        # state delta before o: it only needs vp, so it
                        # fills the PE queue while bm_c is still in flight
                        for fc in range(4):
                            nc.tensor.matmul(
                                d_del[:, 65 * fc:65 * fc + 65],
                                kp_t_v[:, c, 128 * fc:128 * (fc + 1)],
                                vp[:],
                                start=True, stop=True)
                        # out_c = B^T @ V' (intra) + qp_c @ S (inter)
                        o_ps = ops.tile([128, 65], f32, tag="o", name="o")
                        nc.tensor.matmul(o_ps[:], bm[:], vp[:],
                                         start=True, stop=False)
                        for fc in range(4):
                            nc.tensor.matmul(
                                o_ps[:],
                                qp[p][fc][:, 129 * c:129 * c + 128],
                                st_cur[:, 65 * fc:65 * fc + 65],
                                start=False, stop=(fc == 3))
                        # normalize: out / (denom + eps*qsum)
                        rcp = miscp.tile([128, 1], f32, tag=f"rcp_{p}",
                                         name=f"rcp_{p}")
                        nc.vector.reciprocal(out=rcp[:], in_=o_ps[:, 64:65])
                        nc.vector.tensor_scalar_mul(
                            out=on_all[:, 256 * c + 64 * m:
                                       256 * c + 64 * (m + 1)],
                            in0=o_ps[:, 0:64],
                            scalar1=rcp[:])
                        # out-of-place state add into the other ping-pong
                        # buffer: only a read of st_cur, so it runs as soon
                        # as delta lands
                        st_new = stp.tile([128, 65 * 4], bf16, tag=f"st_{p}",
                                          name=f"st_{p}")
                        with nc.allow_low_precision(reason="bf16 state"):
                            nc.vector.tensor_add(out=st_new[:], in0=st_cur[:],
                                                 in1=d_del[:])
                        st_cur = st_new

            # output DMA split by token quarter: each store fires as soon
            # as the last head finishes that quarter of the scan
            for g in range(4):
                sl = slice(1024 * g, 1024 * (g + 1))
                nc.sync.dma_start(out=part_d[:, sl], in_=on_all[:, sl])

    nc.compile()
    return nc


def _host_prep(h, w_qkv, w_o, proj_matrix):
    """Build per-core input maps (bf16, packed)."""
    import ml_dtypes
    bf16 = ml_dtypes.bfloat16

    projs = (proj_matrix * (DH ** -0.25)).astype(np.float32)  # (64, 256)
    masku = (np.arange(128)[:, None] <= np.arange(128)[None, :])

    in_maps = []
    for core in range(N_CORES):
        b, hg = core // 2, core % 2
        heads = [HPC * hg + mm for mm in range(HPC)]
        # hT packed [p, kc, t]: original row = 128*kc + p
        hT = h[:, b, :].T.reshape(4, 128, L).transpose(1, 0, 2).reshape(
            128, 4 * L)
        wqkT = np.empty((DM, 128 * HPC), np.float32)
        wvT = np.empty((DM, 64 * HPC), np.float32)
        for mm, hh in enumerate(heads):
            blk = w_qkv[192 * hh:192 * (hh + 1)]  # (192, DM) = [q64,k64,v64]
            wqkT[:, 128 * mm:128 * mm + 64] = blk[0:64].T
            wqkT[:, 128 * mm + 64:128 * (mm + 1)] = blk[64:128].T
            wvT[:, 64 * mm:64 * (mm + 1)] = blk[128:192].T
        blob = np.zeros((128, BLOB_COLS), np.float32)
        blob[:, BLOB_WQK:BLOB_WQK + 2048] = wqkT.reshape(
            4, 128, 512).transpose(1, 0, 2).reshape(128, 2048)
        blob[:, BLOB_WV:BLOB_WV + 1024] = wvT.reshape(
            4, 128, 256).transpose(1, 0, 2).reshape(128, 1024)
        blob[:, BLOB_PROJ:BLOB_PROJ + 256] = np.concatenate([projs, projs], 0)
        blob[:, BLOB_MASK:BLOB_MASK + 128] = masku
        in_maps.append({
            "hT": hT.astype(bf16),
            "blob": blob.astype(bf16),
        })
    return in_maps


def kernel(h, w_qkv, w_o, ln_gamma, ln_beta, proj_matrix):
    from concourse.bass_utils import run_bass_kernel_spmd

    h = np.asarray(h, np.float32)
    w_qkv = np.asarray(w_qkv, np.float32)
    w_o = np.asarray(w_o, np.float32)
    ln_gamma = np.asarray(ln_gamma, np.float32)
    ln_beta = np.asarray(ln_beta, np.float32)
    proj_matrix = np.asarray(proj_matrix, np.float32)

    if "nc" not in _CACHE:
        _CACHE["nc"] = _build_nc()
    nc = _CACHE["nc"]

    in_maps = _host_prep(h, w_qkv, w_o, proj_matrix)
    res = run_bass_kernel_spmd(nc, in_maps, core_ids=list(range(N_CORES)))

    woT = (w_o.T * SCALE).astype(np.float32)  # (H*DH, DM)
    out = np.empty((L, B, DM), np.float32)
    for b in range(B):
        halves = []
        for hg in range(2):
            raw = np.asarray(res.results[2 * b + hg]["part"])
            on = raw.astype(np.float32).reshape(128, NCH, 4, DH)
            halves.append(on.transpose(1, 0, 2, 3).reshape(L, 4 * DH))
        attn = np.concatenate(halves, axis=1) @ woT  # (L, DM)
        x = h[:, b, :] + attn
        mu = x.mean(-1, keepdims=True)
        var = ((x - mu) ** 2).mean(-1, keepdims=True)
        out[:, b, :] = (x - mu) / np.sqrt(var + EPS_LN) * ln_gamma + ln_beta
    return out
